# revision 16
# baseline (speedup 1.0000x reference)
"""Distributed Trainium2 kernel for the LN->silu->QKV(+LN on q,k)->attention->silu->proj block.

Sharding: sequence-parallel over 8 NeuronCores. Each core owns 512 of the 4096
tokens; both LayerNorms are per-token so they stay local. The only collectives
are AllGathers of K^T and V_aug so every core can attend over all 4096 keys.

Device layout conventions (per core):
  natural    = [token partitions, feature free]
  transposed = [feature partitions, token free]
Scores are computed transposed (S^T = [key, query]) so the softmax denominators
come free from the P@V matmul: V is augmented with a ones column, so the AV
accumulator row 64 is sum_k P. exp() needs no max subtraction: q,k are
LayerNorm outputs and q is scaled by inner^-0.5, so |scores| <~ 3.

Precision: all matmul operands are bf16 (fp32 matmuls cost 2 PE passes), all
accumulation fp32 in PSUM, LayerNorm statistics fp32. silu is computed as
z*(1+tanh(z/2)) = 2*silu(z), with the 0.5 folded into the host-scaled weights
(exact in bf16), so the ACT engine only ever needs ln/exp/tanh tables.
"""

import sys
import numpy as np

sys.path.insert(0, "/opt/trn_rl_repo")

import concourse.bacc as bacc  # noqa: E402
import concourse.tile as tile  # noqa: E402
from concourse import mybir, masks  # noqa: E402
from concourse.bass_utils import run_bass_kernel_spmd  # noqa: E402

FP = mybir.dt.float32
BF = mybir.dt.bfloat16
AF = mybir.ActivationFunctionType
ALU = mybir.AluOpType

NC = 8          # cores
P = 128         # partitions
N = 4096        # sequence
C = 512         # channels
INNER = 512     # heads * dim_head
H = 8           # heads
D = 64          # dim per head
TLOC = N // NC  # tokens per core (512)
NJ = TLOC // P  # token tiles per core (4)
NCH = C // P    # channel chunks (4)
EPS = 1e-5
VW = D + 1      # augmented v width (65)
VWP = H * VW    # padded v row width (520)
BLK = 3         # key chunks per score block (3 PSUM banks per head)

_CACHE = {}


def _ln_batched(nc, pool, src_ap, tag, eps_ap):
    """Batched LayerNorm stats over src_ap [128, NJ, 512] (bn_stats per j —
    its free dim is capped at 512 — but ln/exp/fixup batched across j).

    Returns (rs4, nmr4) [128, NJ]: rsqrt(var+eps) and -mean*rsqrt, computed as
    exp(-0.5*ln(var+eps)) to stay in the ln/exp ACT table set."""
    stats = pool.tile([P, NJ, 6], FP, tag=f"{tag}_st", name=f"{tag}_st")
    for j in range(NJ):
        nc.vector.bn_stats(stats[:, j, :], src_ap[:, j, :])
    aggr = pool.tile([P, NJ, 2], FP, tag=f"{tag}_ag", name=f"{tag}_ag")
    for j in range(NJ):
        nc.vector.bn_aggr(aggr[:, j, :], stats[:, j, :])
    lnv = pool.tile([P, NJ], FP, tag=f"{tag}_lv", name=f"{tag}_lv")
    nc.scalar.activation(lnv[:], aggr[:, :, 1], AF.Ln, bias=eps_ap, scale=1.0)
    rs4 = pool.tile([P, NJ], FP, tag=f"{tag}_rs", name=f"{tag}_rs")
    nc.scalar.activation(rs4[:], lnv[:], AF.Exp, bias=0.0, scale=-0.5)
    nmr4 = pool.tile([P, NJ], FP, tag=f"{tag}_nm", name=f"{tag}_nm")
    nc.vector.scalar_tensor_tensor(
        nmr4[:], aggr[:, :, 0], -1.0, rs4[:], ALU.mult, ALU.mult
    )
    return rs4, nmr4


def _silu2(nc, pool, out_ap, z_ap, tag):
    """out = z*(1+tanh(z/2)) = 2*silu(z); the 0.5 is folded into weights."""
    th = pool.tile(list(z_ap.shape), FP, tag=f"{tag}_th", name=f"{tag}_th")
    nc.scalar.activation(th[:], z_ap, AF.Tanh, bias=0.0, scale=0.5)
    nc.vector.scalar_tensor_tensor(out_ap, th[:], 1.0, z_ap, ALU.add, ALU.mult)


def build_graph():
    nc = bacc.Bacc("TRN2", target_bir_lowering=False, debug=False, num_devices=NC)

    x_in = nc.dram_tensor("x", [TLOC, C], FP, kind="ExternalInput")
    w_in = {}
    for nm in ("wq", "wk", "wv", "wo"):
        w_in[nm] = nc.dram_tensor(nm, [C, C], BF, kind="ExternalInput")
    row_in = {}
    for nm in ("bq", "bk", "bv", "bo"):
        row_in[nm] = nc.dram_tensor(nm, [1, C], BF, kind="ExternalInput")
    for nm in ("gq", "beq", "gk", "bek"):
        row_in[nm] = nc.dram_tensor(nm, [1, C], FP, kind="ExternalInput")
    out_ext = nc.dram_tensor("out", [TLOC, C], FP, kind="ExternalOutput")

    # DRAM bounce buffers for the collectives (bf16)
    k_bounce = nc.dram_tensor("k_bounce", [TLOC, TLOC], BF)        # k^T local
    v_bounce = nc.dram_tensor("v_bounce", [TLOC, VWP], BF)         # v_aug local
    k_gath = nc.dram_tensor("k_gath", [N, TLOC], BF, addr_space="Shared")
    v_gath = nc.dram_tensor("v_gath", [N, VWP], BF, addr_space="Shared")

    with tile.TileContext(nc) as tc:
        with tc.tile_pool(name="persist", bufs=1) as pers:
            ident = pers.tile([P, P], BF)
            masks.make_identity(nc, ident[:])
            ones_r = pers.tile([1, P], BF)
            nc.vector.memset(ones_r[:], 1.0)
            epst = pers.tile([P, 1], FP)
            nc.vector.memset(epst[:], EPS)

            wo_t = pers.tile([P, NCH, C], BF)
            for cc in range(NCH):
                nc.sync.dma_start(
                    out=wo_t[:, cc, :], in_=w_in["wo"][cc * P:(cc + 1) * P, :]
                )
            bo_t = pers.tile([1, C], BF)
            nc.sync.dma_start(out=bo_t[:], in_=row_in["bo"][:])

            # qT: [inner partitions, token free]; chunk c covers heads 2c,2c+1
            qT = [pers.tile([P, NJ, P], BF, tag=f"qT{c}", name=f"qT{c}")
                  for c in range(NCH)]
            soT = [pers.tile([P, NJ, P], BF, tag=f"soT{c}", name=f"soT{c}")
                   for c in range(NCH)]

            # ---------------- phase 1: local projections ----------------
            with tc.tile_pool(name="ph1", bufs=1) as ph1, \
                 tc.tile_pool(name="ph1ps", bufs=1, space="PSUM") as ph1ps, \
                 tc.tile_pool(name="ph1ps2", bufs=2, space="PSUM") as ph1ps2, \
                 tc.tile_pool(name="ph1sm", bufs=2) as ph1sm:
                with tc.high_priority(offset=10000):
                    wts = {}
                    for nm in ("wq", "wk", "wv"):
                        wts[nm] = ph1.tile([P, NCH, C], BF, tag=f"t_{nm}",
                                           name=f"t_{nm}")
                        for cc in range(NCH):
                            nc.sync.dma_start(
                                out=wts[nm][:, cc, :],
                                in_=w_in[nm][cc * P:(cc + 1) * P, :],
                            )
                    rows = {}
                    for nm in ("bq", "bk", "bv"):
                        rows[nm] = ph1.tile([1, C], BF, tag=f"r_{nm}", name=f"r_{nm}")
                        nc.sync.dma_start(out=rows[nm][:], in_=row_in[nm][:])
                    cols = {}
                    for nm in ("gq", "beq", "gk", "bek"):
                        cols[nm] = ph1.tile([P, NCH], FP, tag=f"c_{nm}",
                                            name=f"c_{nm}")
                        nc.sync.dma_start(
                            out=cols[nm][:],
                            in_=row_in[nm][0, :].rearrange("(c p) -> p c", p=P),
                        )

                    xt = ph1.tile([P, NJ, C], FP)
                    for j in range(NJ):
                        nc.sync.dma_start(out=xt[:, j, :],
                                          in_=x_in[j * P:(j + 1) * P, :])

                    # s = 2*silu(LN(x)) -> bf16 (0.5 folded into wq/wk/wv)
                    rs4, nmr4 = _ln_batched(nc, ph1sm, xt[:], "x", epst[:, 0:1])
                    st = ph1.tile([P, NJ, C], BF)
                    for j in range(NJ):
                        zn = ph1sm.tile([P, C], FP, tag="zn", name="zn")
                        nc.vector.tensor_scalar(
                            zn[:], xt[:, j, :], rs4[:, j:j + 1], nmr4[:, j:j + 1],
                            ALU.mult, ALU.add,
                        )
                        _silu2(nc, ph1sm, st[:, j, :], zn[:], "sx")

                    # transpose s -> sT [c partitions, tokens]
                    sT = ph1.tile([P, NCH, NJ, P], BF)
                    for cc in range(NCH):
                        ptr = ph1ps2.tile([P, NJ, P], BF, tag="tr")
                        for j in range(NJ):
                            nc.tensor.transpose(
                                ptr[:, j, :], st[:, j, cc * P:(cc + 1) * P], ident[:]
                            )
                        nc.vector.tensor_copy(sT[:, cc], ptr[:])

                def proj(nm):
                    """q/k/v projection into one 4-bank psum tile [128, NJ, C]."""
                    pq = ph1ps.tile([P, NJ, C], FP, tag="proj", name="proj")
                    for j in range(NJ):
                        for cc in range(NCH):
                            nc.tensor.matmul(
                                pq[:, j, :], sT[:, cc, j, :], wts[nm][:, cc, :],
                                start=(cc == 0), stop=False,
                            )
                        nc.tensor.matmul(
                            pq[:, j, :], ones_r[:], rows[f"b{nm[1]}"][:],
                            start=False, stop=True,
                        )
                    return pq

                def norm_transpose(pq, tag, gcol, becol, dstf):
                    """Batched LN stats on psum [128,NJ,C]; per-j normalize to
                    bf16; transpose; affine fused on the psum->sbuf copy."""
                    rsq, nmq = _ln_batched(nc, ph1sm, pq[:], tag, epst[:, 0:1])
                    for j in range(NJ):
                        yn = ph1sm.tile([P, C], BF, tag=f"{tag}n", name=f"{tag}n")
                        nc.vector.tensor_scalar(
                            yn[:], pq[:, j, :], rsq[:, j:j + 1], nmq[:, j:j + 1],
                            ALU.mult, ALU.add,
                        )
                        for cc in range(NCH):
                            ptr = ph1ps2.tile([P, P], BF, tag="trk", name="trk")
                            nc.tensor.transpose(
                                ptr[:], yn[:, cc * P:(cc + 1) * P], ident[:]
                            )
                            nc.vector.tensor_scalar(
                                dstf(cc, j), ptr[:], gcol[:, cc:cc + 1],
                                becol[:, cc:cc + 1], ALU.mult, ALU.add,
                            )

                # ---- k path first, gather ASAP ----
                kT_loc = ph1.tile([P, NCH, NJ, P], BF)   # k^T local
                vaug = ph1.tile([P, NJ, H, VW], BF)      # v augmented, natural
                with tc.high_priority(offset=10000):
                    pk = proj("wk")
                    norm_transpose(pk, "k", cols["gk"], cols["bek"],
                                   lambda cc, jj: kT_loc[:, cc, jj, :])
                    for cc in range(NCH):
                        nc.sync.dma_start(
                            out=k_bounce[cc * P:(cc + 1) * P, :],
                            in_=kT_loc[:, cc],
                        )
                    nc.gpsimd.collective_compute(
                        "AllGather", ALU.bypass,
                        replica_groups=[list(range(NC))],
                        ins=[k_bounce[:].opt()],
                        outs=[k_gath[:].opt()],
                    )

                    pv = proj("wv")
                    for j in range(NJ):
                        nc.vector.memset(vaug[:, j, :, D:VW], 1.0)
                        nc.vector.tensor_copy(
                            vaug[:, j, :, 0:D],
                            pv[:, j, :].rearrange("p (h d) -> p h d", h=H),
                        )
                        nc.sync.dma_start(
                            out=v_bounce[j * P:(j + 1) * P, :], in_=vaug[:, j]
                        )
                    nc.gpsimd.collective_compute(
                        "AllGather", ALU.bypass,
                        replica_groups=[list(range(NC))],
                        ins=[v_bounce[:].opt()],
                        outs=[v_gath[:].opt()],
                    )

                # ---- q path (overlaps the collectives); gq/beq pre-scaled ----
                pq = proj("wq")
                norm_transpose(pq, "q", cols["gq"], cols["beq"],
                               lambda cc, jj: qT[cc][:, jj, :])

            # ---------------- phase 2: attention ----------------
            CHUNKS = N // P  # 32 key chunks
            blocks = [list(range(i, min(i + BLK, CHUNKS)))
                      for i in range(0, CHUNKS, BLK)]

            with tc.tile_pool(name="att", bufs=3) as att, \
                 tc.tile_pool(name="attps", bufs=1, space="PSUM") as attps, \
                 tc.tile_pool(name="attsm", bufs=2) as attsm:
                for pair in range(H // 2):
                    h0 = 2 * pair
                    # k^T rows for this head pair across all ranks: [128, 8, 512]
                    ktp = att.tile([P, NC, TLOC], BF, tag="ktp", name="ktp")
                    for r in range(NC):
                        nc.sync.dma_start(
                            out=ktp[:, r, :],
                            in_=k_gath[r * TLOC + pair * P:
                                       r * TLOC + (pair + 1) * P, :],
                        )
                    # v_aug for both heads: [128, 32, 2*VW]
                    # issued on the gpsimd queue (after the v collective) so the
                    # sync queue's k-side DMAs never stall behind the v gather
                    vap = att.tile([P, CHUNKS, 2 * VW], BF, tag="vap", name="vap")
                    for r in range(NC):
                        nc.gpsimd.dma_start(
                            out=vap[:, 4 * r:4 * r + 4, :],
                            in_=v_gath[
                                r * TLOC:(r + 1) * TLOC, h0 * VW:(h0 + 2) * VW,
                            ].rearrange("(jj p) w -> p jj w", p=P),
                        )

                    oacc = [
                        attps.tile([VW, TLOC], FP, tag=f"oacc{i}", name=f"oacc{i}")
                        for i in range(2)
                    ]
                    qTp = qT[pair]
                    for blk in blocks:
                        nb = len(blk)
                        psc = [
                            attps.tile([P, BLK, TLOC], FP, tag=f"sc{i}",
                                       name=f"sc{i}")
                            for i in range(2)
                        ]
                        for i, cc in enumerate(blk):
                            r, jj = cc // 4, cc % 4
                            for hh in range(2):
                                o = D * hh
                                nc.tensor.matmul(
                                    psc[hh][:, i, :],
                                    ktp[o:o + D, r, jj * P:(jj + 1) * P],
                                    qTp[o:o + D, :, :],
                                    start=True, stop=True,
                                )
                        pex = [
                            attsm.tile([P, BLK, TLOC], BF, tag=f"pex{i}",
                                       name=f"pex{i}")
                            for i in range(2)
                        ]
                        for hh in range(2):
                            nc.scalar.activation(
                                pex[hh][:, 0:nb, :], psc[hh][:, 0:nb, :], AF.Exp
                            )
                        for i, cc in enumerate(blk):
                            for hh in range(2):
                                nc.tensor.matmul(
                                    oacc[hh][:],
                                    vap[:, cc, hh * VW:(hh + 1) * VW],
                                    pex[hh][:, i, :],
                                    start=(cc == 0), stop=(cc == CHUNKS - 1),
                                )

                    for hh in range(2):
                        # sums -> bf16 row, replicate to 64 partitions via a K=1
                        # matmul, 1/x, then normalize and 2*silu into soT.
                        smb = attsm.tile([1, TLOC], BF, tag=f"smb{hh}",
                                         name=f"smb{hh}")
                        nc.vector.tensor_copy(smb[:], oacc[hh][D:VW, :])
                        srep = attps.tile([D, TLOC], FP, tag=f"sc{hh}",
                                          name=f"srep{hh}")
                        nc.tensor.matmul(srep[:], ones_r[:, 0:D], smb[:],
                                         start=True, stop=True)
                        ssb = attsm.tile([D, TLOC], FP, tag=f"ssb{hh}",
                                         name=f"ssb{hh}")
                        nc.vector.tensor_copy(ssb[:], srep[:])
                        rrep = attsm.tile([D, TLOC], FP, tag=f"rr{hh}",
                                          name=f"rr{hh}")
                        nc.vector.reciprocal_approx_fast(rrep[:], ssb[:])
                        onrm = attsm.tile([D, TLOC], FP, tag=f"on{hh}",
                                          name=f"on{hh}")
                        nc.vector.tensor_mul(onrm[:], oacc[hh][0:D, :], rrep[:])
                        o = D * hh
                        _silu2(nc, attsm, soT[pair][o:o + D, :, :], onrm[:],
                               f"so{hh}")

            # ---------------- phase 3: output projection ----------------
            with tc.tile_pool(name="ph3ps", bufs=2, space="PSUM") as ph3ps, \
                 tc.tile_pool(name="ph3", bufs=2) as ph3:
                for j in range(NJ):
                    po = ph3ps.tile([P, C], FP, tag="po", name="po")
                    for cc in range(NCH):
                        nc.tensor.matmul(
                            po[:], soT[cc][:, j, :], wo_t[:, cc, :],
                            start=(cc == 0), stop=False,
                        )
                    nc.tensor.matmul(po[:], ones_r[:], bo_t[:],
                                     start=False, stop=True)
                    osb = ph3.tile([P, C], FP, tag="osb", name="osb")
                    nc.vector.tensor_copy(osb[:], po[:])
                    nc.sync.dma_start(out=out_ext[j * P:(j + 1) * P, :], in_=osb[:])

    nc.compile()
    return nc


def prepare_in_maps(inputs):
    """Host-side preprocessing: bf16 weight casts (with the silu 0.5 fold),
    query-scale fold into g/be, per-core x shards."""
    import ml_dtypes
    bf16 = ml_dtypes.bfloat16

    x = np.asarray(inputs["x"], dtype=np.float32)
    assert x.shape == (1, N, C)
    scale = np.float32(INNER ** -0.5)

    def wb(a, mul):
        return np.ascontiguousarray(
            (np.asarray(a, np.float32) * mul).astype(bf16)
        )

    def rowb(a):
        return np.ascontiguousarray(
            np.asarray(a, np.float32).reshape(1, C).astype(bf16)
        )

    def rowf(a):
        return np.ascontiguousarray(np.asarray(a, np.float32).reshape(1, C))

    common = {
        # 0.5 folds: s and silu(o) are computed as 2*silu(.)
        "wq": wb(inputs["w_q"], 0.5),
        "wk": wb(inputs["w_k"], 0.5),
        "wv": wb(inputs["w_v"], 0.5),
        "wo": wb(inputs["w_o"], 0.5),
        "bq": rowb(inputs["b_q"]),
        "bk": rowb(inputs["b_k"]),
        "bv": rowb(inputs["b_v"]),
        "bo": rowb(inputs["b_o"]),
        "gq": rowf(np.asarray(inputs["g_q"], np.float32) * scale),
        "beq": rowf(np.asarray(inputs["be_q"], np.float32) * scale),
        "gk": rowf(inputs["g_k"]),
        "bek": rowf(inputs["be_k"]),
    }
    in_maps = []
    for r in range(NC):
        m = dict(common)
        m["x"] = np.ascontiguousarray(x[0, r * TLOC:(r + 1) * TLOC, :])
        in_maps.append(m)
    return in_maps


def kernel(**inputs):
    x = np.asarray(inputs["x"], dtype=np.float32)
    B = x.shape[0]
    if "nc" not in _CACHE:
        _CACHE["nc"] = build_graph()
    nc = _CACHE["nc"]
    in_maps = prepare_in_maps(inputs)
    res = run_bass_kernel_spmd(nc, in_maps, core_ids=list(range(NC)))
    out = np.concatenate([res.results[r]["out"] for r in range(NC)], axis=0)
    return out.reshape(B, N, C)


if __name__ == "__main__":
    sys.path.insert(0, "/root/problem")
    import reference

    inputs = {k: np.asarray(v) for k, v in reference.setup_inputs().items()}
    expected = np.asarray(reference.reference(**reference.setup_inputs()))
    actual = kernel(**inputs)
    err = np.linalg.norm(actual - expected) / np.linalg.norm(expected)
    print("Relative error:", err)


# revision 17
# speedup vs baseline: 1.0586x; 1.0586x over previous
"""Distributed Trainium2 kernel for the LN->silu->QKV(+LN on q,k)->attention->silu->proj block.

Sharding: sequence-parallel over 8 NeuronCores. Each core owns 512 of the 4096
tokens; both LayerNorms are per-token so they stay local. The only collectives
are AllGathers of K^T and V_aug so every core can attend over all 4096 keys.

Device layout conventions (per core):
  natural    = [token partitions, feature free]
  transposed = [feature partitions, token free]
Scores are computed transposed (S^T = [key, query]) so the softmax denominators
come free from the P@V matmul: V is augmented with a ones column, so the AV
accumulator row 64 is sum_k P. exp() needs no max subtraction: q,k are
LayerNorm outputs and q is scaled by inner^-0.5, so |scores| <~ 3.

Precision: all matmul operands are bf16 (fp32 matmuls cost 2 PE passes), all
accumulation fp32 in PSUM, LayerNorm statistics fp32. silu is computed as
z*(1+tanh(z/2)) = 2*silu(z), with the 0.5 folded into the host-scaled weights
(exact in bf16), so the ACT engine only ever needs ln/exp/tanh tables.
"""

import sys
import numpy as np

sys.path.insert(0, "/opt/trn_rl_repo")

import concourse.bacc as bacc  # noqa: E402
import concourse.tile as tile  # noqa: E402
from concourse import mybir, masks  # noqa: E402
from concourse.bass_utils import run_bass_kernel_spmd  # noqa: E402

FP = mybir.dt.float32
BF = mybir.dt.bfloat16
AF = mybir.ActivationFunctionType
ALU = mybir.AluOpType

NC = 8          # cores
P = 128         # partitions
N = 4096        # sequence
C = 512         # channels
INNER = 512     # heads * dim_head
H = 8           # heads
D = 64          # dim per head
TLOC = N // NC  # tokens per core (512)
NJ = TLOC // P  # token tiles per core (4)
NCH = C // P    # channel chunks (4)
EPS = 1e-5
VW = D + 1      # augmented v width (65)
VWP = H * VW    # padded v row width (520)
BLK = 3         # key chunks per score block (3 PSUM banks per head)

_CACHE = {}


def _ln_batched(nc, pool, src_ap, tag, eps_ap):
    """Batched LayerNorm stats over src_ap [128, NJ, 512] (bn_stats per j —
    its free dim is capped at 512 — but ln/exp/fixup batched across j).

    Returns (rs4, nmr4) [128, NJ]: rsqrt(var+eps) and -mean*rsqrt, computed as
    exp(-0.5*ln(var+eps)) to stay in the ln/exp ACT table set."""
    stats = pool.tile([P, NJ, 6], FP, tag=f"{tag}_st", name=f"{tag}_st")
    for j in range(NJ):
        nc.vector.bn_stats(stats[:, j, :], src_ap[:, j, :])
    aggr = pool.tile([P, NJ, 2], FP, tag=f"{tag}_ag", name=f"{tag}_ag")
    for j in range(NJ):
        nc.vector.bn_aggr(aggr[:, j, :], stats[:, j, :])
    lnv = pool.tile([P, NJ], FP, tag=f"{tag}_lv", name=f"{tag}_lv")
    nc.scalar.activation(lnv[:], aggr[:, :, 1], AF.Ln, bias=eps_ap, scale=1.0)
    rs4 = pool.tile([P, NJ], FP, tag=f"{tag}_rs", name=f"{tag}_rs")
    nc.scalar.activation(rs4[:], lnv[:], AF.Exp, bias=0.0, scale=-0.5)
    nmr4 = pool.tile([P, NJ], FP, tag=f"{tag}_nm", name=f"{tag}_nm")
    nc.vector.scalar_tensor_tensor(
        nmr4[:], aggr[:, :, 0], -1.0, rs4[:], ALU.mult, ALU.mult
    )
    return rs4, nmr4


def _silu2(nc, pool, out_ap, z_ap, tag):
    """out = z*(1+tanh(z/2)) = 2*silu(z); the 0.5 is folded into weights."""
    th = pool.tile(list(z_ap.shape), FP, tag=f"{tag}_th", name=f"{tag}_th")
    nc.scalar.activation(th[:], z_ap, AF.Tanh, bias=0.0, scale=0.5)
    nc.vector.scalar_tensor_tensor(out_ap, th[:], 1.0, z_ap, ALU.add, ALU.mult)


def build_graph():
    nc = bacc.Bacc("TRN2", target_bir_lowering=False, debug=False, num_devices=NC)

    x_in = nc.dram_tensor("x", [TLOC, C], FP, kind="ExternalInput")
    w_in = {}
    for nm in ("wq", "wk", "wv", "wo"):
        w_in[nm] = nc.dram_tensor(nm, [C, C], BF, kind="ExternalInput")
    row_in = {}
    for nm in ("bq", "bk", "bv", "bo"):
        row_in[nm] = nc.dram_tensor(nm, [1, C], BF, kind="ExternalInput")
    for nm in ("gq", "beq", "gk", "bek"):
        row_in[nm] = nc.dram_tensor(nm, [1, C], FP, kind="ExternalInput")
    out_ext = nc.dram_tensor("out", [TLOC, C], FP, kind="ExternalOutput")

    # DRAM bounce buffers for the collectives (bf16)
    k_bounce = nc.dram_tensor("k_bounce", [TLOC, TLOC], BF)        # k^T local
    v_bounce = nc.dram_tensor("v_bounce", [TLOC, VWP], BF)         # v_aug local
    k_gath = nc.dram_tensor("k_gath", [N, TLOC], BF, addr_space="Shared")
    v_gath = nc.dram_tensor("v_gath", [N, VWP], BF, addr_space="Shared")

    with tile.TileContext(nc) as tc:
        with tc.tile_pool(name="persist", bufs=1) as pers:
            ident = pers.tile([P, P], BF)
            masks.make_identity(nc, ident[:])
            ones_r = pers.tile([1, P], BF)
            nc.vector.memset(ones_r[:], 1.0)
            epst = pers.tile([P, 1], FP)
            nc.vector.memset(epst[:], EPS)

            wo_t = pers.tile([P, NCH, C], BF)
            for cc in range(NCH):
                nc.sync.dma_start(
                    out=wo_t[:, cc, :], in_=w_in["wo"][cc * P:(cc + 1) * P, :]
                )
            bo_t = pers.tile([1, C], BF)
            nc.sync.dma_start(out=bo_t[:], in_=row_in["bo"][:])

            # qT: [inner partitions, token free]; chunk c covers heads 2c,2c+1
            qT = [pers.tile([P, NJ, P], BF, tag=f"qT{c}", name=f"qT{c}")
                  for c in range(NCH)]
            soT = [pers.tile([P, NJ, P], BF, tag=f"soT{c}", name=f"soT{c}")
                   for c in range(NCH)]

            # ---------------- phase 1: local projections ----------------
            with tc.tile_pool(name="ph1", bufs=1) as ph1, \
                 tc.tile_pool(name="ph1ps", bufs=1, space="PSUM") as ph1ps, \
                 tc.tile_pool(name="ph1ps2", bufs=2, space="PSUM") as ph1ps2, \
                 tc.tile_pool(name="ph1sm", bufs=2) as ph1sm:
                with tc.high_priority(offset=10000):
                    wts = {}
                    for nm in ("wq", "wk", "wv"):
                        wts[nm] = ph1.tile([P, NCH, C], BF, tag=f"t_{nm}",
                                           name=f"t_{nm}")
                        for cc in range(NCH):
                            nc.sync.dma_start(
                                out=wts[nm][:, cc, :],
                                in_=w_in[nm][cc * P:(cc + 1) * P, :],
                            )
                    rows = {}
                    for nm in ("bq", "bk", "bv"):
                        rows[nm] = ph1.tile([1, C], BF, tag=f"r_{nm}", name=f"r_{nm}")
                        nc.sync.dma_start(out=rows[nm][:], in_=row_in[nm][:])
                    cols = {}
                    for nm in ("gq", "beq", "gk", "bek"):
                        cols[nm] = ph1.tile([P, NCH], FP, tag=f"c_{nm}",
                                            name=f"c_{nm}")
                        nc.sync.dma_start(
                            out=cols[nm][:],
                            in_=row_in[nm][0, :].rearrange("(c p) -> p c", p=P),
                        )

                    xt = ph1.tile([P, NJ, C], FP)
                    for j in range(NJ):
                        nc.sync.dma_start(out=xt[:, j, :],
                                          in_=x_in[j * P:(j + 1) * P, :])

                    # s = 2*silu(LN(x)) -> bf16 (0.5 folded into wq/wk/wv)
                    rs4, nmr4 = _ln_batched(nc, ph1sm, xt[:], "x", epst[:, 0:1])
                    st = ph1.tile([P, NJ, C], BF)
                    for j in range(NJ):
                        zn = ph1sm.tile([P, C], FP, tag="zn", name="zn")
                        nc.vector.tensor_scalar(
                            zn[:], xt[:, j, :], rs4[:, j:j + 1], nmr4[:, j:j + 1],
                            ALU.mult, ALU.add,
                        )
                        _silu2(nc, ph1sm, st[:, j, :], zn[:], "sx")

                    # transpose s -> sT [c partitions, tokens]
                    sT = ph1.tile([P, NCH, NJ, P], BF)
                    for cc in range(NCH):
                        ptr = ph1ps2.tile([P, NJ, P], BF, tag="tr")
                        for j in range(NJ):
                            nc.tensor.transpose(
                                ptr[:, j, :], st[:, j, cc * P:(cc + 1) * P], ident[:]
                            )
                        nc.vector.tensor_copy(sT[:, cc], ptr[:])

                def proj(nm):
                    """q/k/v projection into one 4-bank psum tile [128, NJ, C]."""
                    pq = ph1ps.tile([P, NJ, C], FP, tag="proj", name="proj")
                    for j in range(NJ):
                        for cc in range(NCH):
                            nc.tensor.matmul(
                                pq[:, j, :], sT[:, cc, j, :], wts[nm][:, cc, :],
                                start=(cc == 0), stop=False,
                            )
                        nc.tensor.matmul(
                            pq[:, j, :], ones_r[:], rows[f"b{nm[1]}"][:],
                            start=False, stop=True,
                        )
                    return pq

                def norm_transpose(pq, tag, gcol, becol, dstf):
                    """Batched LN stats on psum [128,NJ,C]; per-j normalize to
                    bf16; transpose; affine fused on the psum->sbuf copy."""
                    rsq, nmq = _ln_batched(nc, ph1sm, pq[:], tag, epst[:, 0:1])
                    for j in range(NJ):
                        yn = ph1sm.tile([P, C], BF, tag=f"{tag}n", name=f"{tag}n")
                        nc.vector.tensor_scalar(
                            yn[:], pq[:, j, :], rsq[:, j:j + 1], nmq[:, j:j + 1],
                            ALU.mult, ALU.add,
                        )
                        for cc in range(NCH):
                            ptr = ph1ps2.tile([P, P], BF, tag="trk", name="trk")
                            nc.tensor.transpose(
                                ptr[:], yn[:, cc * P:(cc + 1) * P], ident[:]
                            )
                            nc.vector.tensor_scalar(
                                dstf(cc, j), ptr[:], gcol[:, cc:cc + 1],
                                becol[:, cc:cc + 1], ALU.mult, ALU.add,
                            )

                # ---- k path first, gather ASAP ----
                kT_loc = ph1.tile([P, NCH, NJ, P], BF)   # k^T local
                vaug = ph1.tile([P, NJ, H, VW], BF)      # v augmented, natural
                with tc.high_priority(offset=10000):
                    pk = proj("wk")
                    norm_transpose(pk, "k", cols["gk"], cols["bek"],
                                   lambda cc, jj: kT_loc[:, cc, jj, :])
                    for cc in range(NCH):
                        nc.sync.dma_start(
                            out=k_bounce[cc * P:(cc + 1) * P, :],
                            in_=kT_loc[:, cc],
                        )
                    nc.gpsimd.collective_compute(
                        "AllGather", ALU.bypass,
                        replica_groups=[list(range(NC))],
                        ins=[k_bounce[:].opt()],
                        outs=[k_gath[:].opt()],
                    )

                    pv = proj("wv")
                    for j in range(NJ):
                        nc.vector.memset(vaug[:, j, :, D:VW], 1.0)
                        nc.vector.tensor_copy(
                            vaug[:, j, :, 0:D],
                            pv[:, j, :].rearrange("p (h d) -> p h d", h=H),
                        )
                        nc.sync.dma_start(
                            out=v_bounce[j * P:(j + 1) * P, :], in_=vaug[:, j]
                        )
                    nc.gpsimd.collective_compute(
                        "AllGather", ALU.bypass,
                        replica_groups=[list(range(NC))],
                        ins=[v_bounce[:].opt()],
                        outs=[v_gath[:].opt()],
                    )

                # ---- q path (overlaps the collectives); gq/beq pre-scaled ----
                pq = proj("wq")
                norm_transpose(pq, "q", cols["gq"], cols["beq"],
                               lambda cc, jj: qT[cc][:, jj, :])

            # ---------------- phase 2: attention ----------------
            CHUNKS = N // P  # 32 key chunks
            blocks = [list(range(i, min(i + BLK, CHUNKS)))
                      for i in range(0, CHUNKS, BLK)]

            with tc.tile_pool(name="att", bufs=2) as att, \
                 tc.tile_pool(name="attps", bufs=1, space="PSUM") as attps, \
                 tc.tile_pool(name="attsm", bufs=2) as attsm:
                for pair in range(H // 2):
                    h0 = 2 * pair
                    # k^T rows for this head pair across all ranks: [128, 8, 512]
                    ktp = att.tile([P, NC, TLOC], BF, tag="ktp", name="ktp")
                    for r in range(NC):
                        nc.sync.dma_start(
                            out=ktp[:, r, :],
                            in_=k_gath[r * TLOC + pair * P:
                                       r * TLOC + (pair + 1) * P, :],
                        )
                    # v_aug for both heads: [128, 32, 2*VW]
                    vap = att.tile([P, CHUNKS, 2 * VW], BF, tag="vap", name="vap")
                    for r in range(NC):
                        nc.sync.dma_start(
                            out=vap[:, 4 * r:4 * r + 4, :],
                            in_=v_gath[
                                r * TLOC:(r + 1) * TLOC, h0 * VW:(h0 + 2) * VW,
                            ].rearrange("(jj p) w -> p jj w", p=P),
                        )

                    oacc = [
                        attps.tile([VW, TLOC], FP, tag=f"oacc{i}", name=f"oacc{i}")
                        for i in range(2)
                    ]
                    qTp = qT[pair]
                    for blk in blocks:
                        nb = len(blk)
                        psc = [
                            attps.tile([P, BLK, TLOC], FP, tag=f"sc{i}",
                                       name=f"sc{i}")
                            for i in range(2)
                        ]
                        for i, cc in enumerate(blk):
                            r, jj = cc // 4, cc % 4
                            for hh in range(2):
                                o = D * hh
                                nc.tensor.matmul(
                                    psc[hh][:, i, :],
                                    ktp[o:o + D, r, jj * P:(jj + 1) * P],
                                    qTp[o:o + D, :, :],
                                    start=True, stop=True,
                                )
                        pex = [
                            attsm.tile([P, BLK, TLOC], BF, tag=f"pex{i}",
                                       name=f"pex{i}")
                            for i in range(2)
                        ]
                        for hh in range(2):
                            nc.scalar.activation(
                                pex[hh][:, 0:nb, :], psc[hh][:, 0:nb, :], AF.Exp
                            )
                        for i, cc in enumerate(blk):
                            for hh in range(2):
                                nc.tensor.matmul(
                                    oacc[hh][:],
                                    vap[:, cc, hh * VW:(hh + 1) * VW],
                                    pex[hh][:, i, :],
                                    start=(cc == 0), stop=(cc == CHUNKS - 1),
                                )

                    for hh in range(2):
                        # sums -> bf16 row, replicate to 64 partitions via a K=1
                        # matmul, 1/x, then normalize and 2*silu into soT.
                        smb = attsm.tile([1, TLOC], BF, tag=f"smb{hh}",
                                         name=f"smb{hh}")
                        nc.vector.tensor_copy(smb[:], oacc[hh][D:VW, :])
                        srep = attps.tile([D, TLOC], FP, tag=f"sc{hh}",
                                          name=f"srep{hh}")
                        nc.tensor.matmul(srep[:], ones_r[:, 0:D], smb[:],
                                         start=True, stop=True)
                        ssb = attsm.tile([D, TLOC], FP, tag=f"ssb{hh}",
                                         name=f"ssb{hh}")
                        nc.vector.tensor_copy(ssb[:], srep[:])
                        rrep = attsm.tile([D, TLOC], FP, tag=f"rr{hh}",
                                          name=f"rr{hh}")
                        nc.vector.reciprocal_approx_fast(rrep[:], ssb[:])
                        onrm = attsm.tile([D, TLOC], FP, tag=f"on{hh}",
                                          name=f"on{hh}")
                        nc.vector.tensor_mul(onrm[:], oacc[hh][0:D, :], rrep[:])
                        o = D * hh
                        _silu2(nc, attsm, soT[pair][o:o + D, :, :], onrm[:],
                               f"so{hh}")

            # ---------------- phase 3: output projection ----------------
            with tc.tile_pool(name="ph3ps", bufs=2, space="PSUM") as ph3ps, \
                 tc.tile_pool(name="ph3", bufs=2) as ph3:
                for j in range(NJ):
                    po = ph3ps.tile([P, C], FP, tag="po", name="po")
                    for cc in range(NCH):
                        nc.tensor.matmul(
                            po[:], soT[cc][:, j, :], wo_t[:, cc, :],
                            start=(cc == 0), stop=False,
                        )
                    nc.tensor.matmul(po[:], ones_r[:], bo_t[:],
                                     start=False, stop=True)
                    osb = ph3.tile([P, C], FP, tag="osb", name="osb")
                    nc.vector.tensor_copy(osb[:], po[:])
                    nc.sync.dma_start(out=out_ext[j * P:(j + 1) * P, :], in_=osb[:])

    nc.compile()
    return nc


def prepare_in_maps(inputs):
    """Host-side preprocessing: bf16 weight casts (with the silu 0.5 fold),
    query-scale fold into g/be, per-core x shards."""
    import ml_dtypes
    bf16 = ml_dtypes.bfloat16

    x = np.asarray(inputs["x"], dtype=np.float32)
    assert x.shape == (1, N, C)
    scale = np.float32(INNER ** -0.5)

    def wb(a, mul):
        return np.ascontiguousarray(
            (np.asarray(a, np.float32) * mul).astype(bf16)
        )

    def rowb(a):
        return np.ascontiguousarray(
            np.asarray(a, np.float32).reshape(1, C).astype(bf16)
        )

    def rowf(a):
        return np.ascontiguousarray(np.asarray(a, np.float32).reshape(1, C))

    common = {
        # 0.5 folds: s and silu(o) are computed as 2*silu(.)
        "wq": wb(inputs["w_q"], 0.5),
        "wk": wb(inputs["w_k"], 0.5),
        "wv": wb(inputs["w_v"], 0.5),
        "wo": wb(inputs["w_o"], 0.5),
        "bq": rowb(inputs["b_q"]),
        "bk": rowb(inputs["b_k"]),
        "bv": rowb(inputs["b_v"]),
        "bo": rowb(inputs["b_o"]),
        "gq": rowf(np.asarray(inputs["g_q"], np.float32) * scale),
        "beq": rowf(np.asarray(inputs["be_q"], np.float32) * scale),
        "gk": rowf(inputs["g_k"]),
        "bek": rowf(inputs["be_k"]),
    }
    in_maps = []
    for r in range(NC):
        m = dict(common)
        m["x"] = np.ascontiguousarray(x[0, r * TLOC:(r + 1) * TLOC, :])
        in_maps.append(m)
    return in_maps


def kernel(**inputs):
    x = np.asarray(inputs["x"], dtype=np.float32)
    B = x.shape[0]
    if "nc" not in _CACHE:
        _CACHE["nc"] = build_graph()
    nc = _CACHE["nc"]
    in_maps = prepare_in_maps(inputs)
    res = run_bass_kernel_spmd(nc, in_maps, core_ids=list(range(NC)))
    out = np.concatenate([res.results[r]["out"] for r in range(NC)], axis=0)
    return out.reshape(B, N, C)


if __name__ == "__main__":
    sys.path.insert(0, "/root/problem")
    import reference

    inputs = {k: np.asarray(v) for k, v in reference.setup_inputs().items()}
    expected = np.asarray(reference.reference(**reference.setup_inputs()))
    actual = kernel(**inputs)
    err = np.linalg.norm(actual - expected) / np.linalg.norm(expected)
    print("Relative error:", err)
